# revision 20
# baseline (speedup 1.0000x reference)
"""3-layer GCN encoder (GCNConv x4, layers 3+4 fused) on 8 Trainium2 NeuronCores.

Strategy (graph/data parallel, per the edge-cut sharding hint):
  - Nodes partitioned contiguously across 8 cores (NLOC = N/8), degree-balanced
    window assignment (128 dst nodes per window) within each core.
  - Per layer: local transform T = h @ W on PE (bf16), rows scaled by dinv[src],
    written as a strided DRAM table (256B row stride; fp8 payload for layers
    1-2, bf16 for the final fused layer), AllGather -> full table per core.
  - Aggregation: per window group, dma_gather fetches each in-edge's source row
    (64B fp8 / 128B bf16 payload per descriptor -- raw-constructed gather
    instructions allow payload < 256B while the row stride stays 256B).
    Segment-sum runs on PE: per 128-slot tile, one matmul with a
    host-precomputed one-hot selection matrix S (fp8, SBUF-resident, shared by
    all three layers; zero columns at pad slots). Self-loop contributions come
    from the local stage tile via an identity matmul that also initializes the
    PSUM accumulator.
  - Epilogue per window: DVE (psum * dinv[dst] + bias), ReLU+bf16 cast, PE
    transpose into the next layer's feature-major input; final layer DMAs
    [nw, 64] = [mu | logstd] f32 rows to DRAM.
  - Gather indices are int16: rows >= 32768 are fetched by a second gather with
    the table base offset by 32768 rows (edges pre-split into lo/hi groups).

Self-contained: only needs numpy/ml_dtypes/concourse (container-installed).
"""

import os
import sys

if "/opt/trn_rl_repo" not in sys.path:
    sys.path.insert(0, "/opt/trn_rl_repo")

import numpy as np
import ml_dtypes

import concourse.bass as bass
import concourse.bacc as bacc
import concourse.mybir as mybir
import concourse.tile as tile
import concourse.ap_utils as ap_utils
from concourse.bass_utils import run_bass_kernel_spmd

BF16 = ml_dtypes.bfloat16
FP8 = ml_dtypes.float8_e4m3fn
F32 = mybir.dt.float32
BF = mybir.dt.bfloat16
E4 = mybir.dt.float8e4
I16 = mybir.dt.int16

N_CORES = 8
CHUNK = 32768  # int16 index range per gather call

_cache = {}
_last = {}


def last_run(trace=False, **kw):
    """Re-run the last compiled kernel/in_maps (optionally with NTFF tracing)."""
    if "nc" not in _last:
        return None
    return run_bass_kernel_spmd(_last["nc"], _last["in_maps"],
                                core_ids=list(range(N_CORES)), trace=trace, **kw)


def _balance(deg, N, NLOC, W):
    """Deal degree-sorted nodes round-robin into windows, per core."""
    lpos = np.empty(N, np.int64)
    caps = np.full(W, 128, np.int64)
    caps[W - 1] = NLOC - 128 * (W - 1)
    for c in range(N_CORES):
        dl = deg[c * NLOC:(c + 1) * NLOC]
        order_ = np.argsort(-dl, kind="stable")
        fill = np.zeros(W, np.int64)
        wi = 0
        pos = np.empty(NLOC, np.int64)
        for i in range(NLOC):
            while fill[wi % W] >= caps[wi % W]:
                wi += 1
            ww = wi % W
            pos[order_[i]] = ww * 128 + fill[ww]
            fill[ww] += 1
            wi += 1
        lpos[c * NLOC:(c + 1) * NLOC] = pos
    return lpos


def _prep(x, ei, N, NLOC, W, GRP):
    """Host-side edge bookkeeping: slot layout, gather indices, one-hot S."""
    src = ei[0].astype(np.int64)
    dst = ei[1].astype(np.int64)
    E = src.shape[0]
    deg = np.bincount(dst, minlength=N).astype(np.float32) + 1.0  # + self loop

    lpos = _balance(deg, N, NLOC, W)
    grow = (np.arange(N, dtype=np.int64) // NLOC) * NLOC + lpos

    srow = grow[src]
    c = dst // NLOC
    dloc = lpos[dst]
    w = dloc >> 7
    p = dloc & 127
    hi = (srow >= CHUNK).astype(np.int64)
    rel = (srow - hi * CHUNK).astype(np.int16)

    key = (c * W + w) * 2 + hi
    order = np.argsort(key, kind="stable")
    cnt = np.bincount(key, minlength=N_CORES * W * 2).reshape(N_CORES, W, 2)
    nt = (cnt.max(axis=0) + 127) // 128  # [W, 2] tiles per (window, chunk)

    # group/tile layout: per group of GRP windows, [all lo tiles | all hi tiles]
    tile_base = np.zeros((W, 2), np.int64)
    groups = []
    tb = 0
    for g0 in range(0, W, GRP):
        gn = min(GRP, W - g0)
        lo_base = tb
        for w_ in range(g0, g0 + gn):
            tile_base[w_, 0] = tb
            tb += nt[w_, 0]
        hi_base = tb
        for w_ in range(g0, g0 + gn):
            tile_base[w_, 1] = tb
            tb += nt[w_, 1]
        groups.append((g0, gn, int(lo_base), int(hi_base),
                       int(hi_base - lo_base), int(tb - hi_base)))
    T_tot = int(tb)
    P_tot = 128 * T_tot

    gstart = np.concatenate([[0], np.cumsum(cnt.reshape(-1))])
    pos_in_grp = np.arange(E) - gstart[key[order]]
    slot = tile_base[w[order], hi[order]] * 128 + pos_in_grp

    idx_arr = np.zeros((N_CORES, P_tot), np.int16)
    idx_arr[c[order], slot] = rel[order]
    S_arr = np.zeros((N_CORES, 128, P_tot), np.uint8)
    one = np.float32(1.0).astype(FP8).view(np.uint8)
    S_arr[c[order], slot % 128, (slot // 128) * 128 + p[order]] = one

    idx_tiles = []
    for cc in range(N_CORES):
        idx16 = np.ascontiguousarray(idx_arr[cc].reshape(P_tot // 16, 16).T)
        idx_tiles.append(np.ascontiguousarray(np.tile(idx16, (8, 1))))

    meta = {
        "N": N, "NLOC": NLOC, "W": W, "GRP": GRP,
        "nt": nt.tolist(), "tile_base": tile_base.tolist(),
        "groups": groups, "T_tot": T_tot, "P_tot": P_tot,
    }
    return meta, lpos, deg, idx_tiles, S_arr.view(FP8)


def _gather_raw(gp, out_ap, in_ap, idxs_ap, num_idxs, elem_size, elem_step):
    """dma_gather without the elem_size_bytes % 256 == 0 restriction.

    in_ap: DRAM [rows (stride elem_step elems), elem_size elems]
    out_ap: SBUF [128, num_idxs//128, elem_size]; idxs_ap: [128, num_idxs//16].
    """
    assert idxs_ap.dtype == mybir.dt.int16
    assert in_ap.dtype == out_ap.dtype
    assert in_ap.ap[0][0] == elem_step
    assert in_ap.ap[-1][1] == elem_size
    assert num_idxs % 128 == 0
    assert out_ap.ap[0][1] * out_ap.ap[1][1] == num_idxs
    assert ap_utils.ap_is_contiguous(out_ap.ap[1:])
    assert ap_utils.ap_is_contiguous(idxs_ap.ap[1:])
    stride_bytes = elem_step * mybir.dt.size(in_ap.dtype)
    stride_bytes_256 = stride_bytes // 256
    assert stride_bytes_256 * 256 == stride_bytes and 0 < stride_bytes_256 < 256
    return gp.add_instruction(
        mybir.InstDMAGatherAnt(
            name=gp.bass.get_next_instruction_name(),
            ins=[
                *gp.lower_ap_dma(in_ap, for_custom_bir_dma=True),
                gp.lower_ap(idxs_ap),
                gp.lower_val_access(gp.to_reg(num_idxs)),
            ],
            outs=[gp.lower_ap(out_ap)],
            transpose=False,
            num_idxs=num_idxs,
            elem_size=elem_size,
            stride_bytes_256=stride_bytes_256,
            gen_mode=0,
            single_packet=False,
            queue_num=0,
            sbuf_tokens_per_rank=0,
            sbuf_free_dim_per_rank=0,
            sbuf_free_dim_pad_per_rank=0,
            sbuf_byte_offset=0,
        )
    )


def _build(meta, solo):
    """Build the SPMD Bass program (solo=True: 1 core, no collectives)."""
    N, NLOC, W = meta["N"], meta["NLOC"], meta["W"]
    nt = meta["nt"]
    tile_base = meta["tile_base"]
    groups = meta["groups"]
    P_tot = meta["P_tot"]
    NTG_MAX = max(g[4] + g[5] for g in groups)
    AG = mybir.AluOpType
    RG = [list(range(N_CORES))]
    AF = mybir.ActivationFunctionType

    nc = bacc.Bacc("TRN2", target_bir_lowering=False, debug=False,
                   num_devices=1 if solo else N_CORES)

    xT_d = nc.dram_tensor("xT", (128, NLOC), BF, kind="ExternalInput")
    idxs_d = nc.dram_tensor("idxs", (128, P_tot // 16), I16,
                            kind="ExternalInput")
    S_d = nc.dram_tensor("S", (128, P_tot), E4, kind="ExternalInput")
    degp_d = nc.dram_tensor("degp", (128, W), F32, kind="ExternalInput")
    W1_d = nc.dram_tensor("W1", (128, 64), BF, kind="ExternalInput")
    W2_d = nc.dram_tensor("W2", (64, 64), BF, kind="ExternalInput")
    W34_d = nc.dram_tensor("W34", (64, 64), BF, kind="ExternalInput")
    b1_d = nc.dram_tensor("b1b", (128, 64), F32, kind="ExternalInput")
    b2_d = nc.dram_tensor("b2b", (128, 64), F32, kind="ExternalInput")
    b34_d = nc.dram_tensor("b34b", (128, 64), F32, kind="ExternalInput")
    id64_d = nc.dram_tensor("id64", (64, 64), BF, kind="ExternalInput")
    id128_d = nc.dram_tensor("id128", (128, 128), BF, kind="ExternalInput")
    out_d = nc.dram_tensor("out34", (NLOC, 64), F32, kind="ExternalOutput")

    ldt = [E4, E4, BF]                 # per-layer message dtype
    lcols = [256, 256, 128]            # table cols (256B row stride)
    tabl = [nc.dram_tensor(f"tabl{l}", (NLOC, lcols[l]), ldt[l],
                           kind="Internal") for l in range(3)]
    tabf = [nc.dram_tensor(f"tabf{l}", (N, lcols[l]), ldt[l], kind="Internal",
                           addr_space="Shared") for l in range(3)]

    with tile.TileContext(nc) as tc:
        with (
            tc.tile_pool(name="const", bufs=1) as const,
            tc.tile_pool(name="ttp", bufs=2) as ttp,
            tc.tile_pool(name="stg", bufs=2) as stg,
            tc.tile_pool(name="hTp", bufs=1) as hTp,
            tc.tile_pool(name="work", bufs=int(os.environ.get("K_WK", "8"))) as work,
            tc.tile_pool(name="gp", bufs=2) as gp,
            tc.tile_pool(name="psT", bufs=2, space="PSUM") as psT,
            tc.tile_pool(name="psR", bufs=2, space="PSUM") as psR,
            tc.tile_pool(name="psA", bufs=int(os.environ.get("K_PSA", "4")),
                         space="PSUM") as psA,
        ):
            def cload(dram, shape, dt, tag):
                t = const.tile(shape, dt, tag=tag)
                nc.sync.dma_start(t[:], dram[:])
                return t

            W1t = cload(W1_d, [128, 64], BF, "W1t")
            W2t = cload(W2_d, [64, 64], BF, "W2t")
            W34t = cload(W34_d, [64, 64], BF, "W34t")
            b1t = cload(b1_d, [128, 64], F32, "b1t")
            b2t = cload(b2_d, [128, 64], F32, "b2t")
            b34t = cload(b34_d, [128, 64], F32, "b34t")
            id64 = cload(id64_d, [64, 64], BF, "id64")
            id128 = cload(id128_d, [128, 128], BF, "id128")

            degp = cload(degp_d, [128, W], F32, "degp")
            sqp = const.tile([128, W], F32, tag="sqp")
            nc.scalar.activation(sqp[:], degp[:], AF.Sqrt)
            dinvp = const.tile([128, W], F32, tag="dinvp")
            nc.vector.reciprocal(dinvp[:], sqp[:])

            # S/idxs are loaded lazily, paced one aggregation group ahead so
            # the 38us S stream never crowds gathers off the DMA engines.
            idxs = const.tile([128, P_tot // 16], I16, tag="idxs")
            S = const.tile([128, P_tot], E4, tag="S")
            SCH = 1680  # S columns per load piece
            sload = [0]

            def load_S_through(tiles):
                """Issue S/idxs pieces covering slot-tiles [0, tiles)."""
                need = min(tiles * 128, P_tot)
                while sload[0] < need:
                    c0 = sload[0]
                    cn = min(SCH, P_tot - c0)
                    nc.scalar.dma_start(S[:, c0:c0 + cn], S_d[:, c0:c0 + cn])
                    nc.scalar.dma_start(idxs[:, c0 // 16:(c0 + cn) // 16],
                                        idxs_d[:, c0 // 16:(c0 + cn) // 16])
                    sload[0] += cn

            NCH = (NLOC + 511) // 512

            def make_transform(l, hT, K, Wt):
                """Returns (stage, emit_chunk, finish). emit_chunk(j): compute
                T = h @ W for nodes [512j, 512j+cn), dinv[src]-scaled and
                transposed into stage, table rows written strided to DRAM."""
                stage = stg.tile([128, W * 64], ldt[l], tag="stage")

                def emit(j):
                    c0 = 512 * j
                    cn = min(512, NLOC - c0)
                    if hT is None:
                        xc = work.tile([128, 512], BF, tag="xc")
                        nc.sync.dma_start(xc[:, :cn], xT_d[:, c0:c0 + cn])
                        rhs = xc[:K, :cn]
                    else:
                        rhs = hT[j][:K, :cn]
                    ps = psT.tile([64, 512], F32, tag="psT")
                    nc.tensor.matmul(ps[:, :cn], Wt[:K, :], rhs,
                                     start=True, stop=True)
                    TTc = ttp.tile([64, 512], BF, tag="TTc")
                    nc.vector.tensor_copy(TTc[:, :cn], ps[:, :cn])
                    w0, w1 = c0 // 128, min((c0 + cn + 127) // 128, W)
                    for w in range(w0, w1):
                        off = w * 128 - c0
                        nw = min(128, NLOC - w * 128)
                        ptt = psR.tile([128, 128], BF, tag="ptr")
                        nc.tensor.transpose(ptt[:nw, :64], TTc[:, off:off + nw],
                                            id64[:, :])
                        nc.scalar.activation(
                            stage[:nw, 64 * w:64 * w + 64], ptt[:nw, :64],
                            AF.Copy, scale=dinvp[:nw, w:w + 1])
                        if nw < 128:
                            nc.vector.memset(stage[nw:, 64 * w:64 * w + 64], 0.0)
                    if cn == 512:
                        nc.sync.dma_start(
                            tabl[l][c0:c0 + cn, 0:64].rearrange(
                                "(w p) e -> p w e", p=128),
                            stage[:, 64 * w0:64 * w1])
                    else:
                        for w in range(w0, w1):
                            nw = min(128, NLOC - w * 128)
                            nc.sync.dma_start(
                                tabl[l][w * 128:w * 128 + nw, 0:64],
                                stage[:nw, 64 * w:64 * w + 64])

                def finish():
                    if not solo:
                        nc.gpsimd.collective_compute(
                            "AllGather", AG.bypass, replica_groups=RG,
                            ins=[tabl[l][:].opt()], outs=[tabf[l][:].opt()])

                return stage, emit, finish

            def aggregate(l, stage, bias_t, relu, hT_next, nxt=None):
                """nxt = (emit, finish) of the next layer's transform; chunks
                are emitted as soon as their source windows are done."""
                in_lo = tabf[l][0:CHUNK, 0:64]
                in_hi = tabf[l][CHUNK:N, 0:64]
                emitted = 0
                ost = [None]
                if l == 0:
                    load_S_through(groups[0][3] + groups[0][5])
                for gi, (g0, gn, lo_base, hi_base, ntg_lo, ntg_hi) \
                        in enumerate(groups):
                    g = gp.tile([128, NTG_MAX, 64], ldt[l], tag="g")
                    if l == 0 and gi + 1 < len(groups):
                        nxt_g = groups[gi + 1]
                        load_S_through(nxt_g[3] + nxt_g[5])
                    if ntg_lo:
                        _gather_raw(
                            nc.gpsimd, g[:, :ntg_lo, :], in_lo,
                            idxs[:, lo_base * 8:(lo_base + ntg_lo) * 8],
                            ntg_lo * 128, 64, lcols[l])
                    if ntg_hi:
                        _gather_raw(
                            nc.gpsimd, g[:, ntg_lo:ntg_lo + ntg_hi, :], in_hi,
                            idxs[:, hi_base * 8:(hi_base + ntg_hi) * 8],
                            ntg_hi * 128, 64, lcols[l])
                    for w in range(g0, g0 + gn):
                        nw = min(128, NLOC - w * 128)
                        n_lo, n_hi = nt[w][0], nt[w][1]
                        ps = psA.tile([128, 64], F32, tag="psA")
                        nc.tensor.matmul(ps[:], id128[:, :],
                                         stage[:, 64 * w:64 * w + 64],
                                         start=True, stop=(n_lo + n_hi == 0))
                        for t in range(n_lo):
                            tb_ = tile_base[w][0] + t
                            nc.tensor.matmul(
                                ps[:], S[:, tb_ * 128:tb_ * 128 + 128],
                                g[:, tb_ - lo_base, :], start=False,
                                stop=(n_hi == 0 and t == n_lo - 1))
                        for t in range(n_hi):
                            tb_ = tile_base[w][1] + t
                            nc.tensor.matmul(
                                ps[:], S[:, tb_ * 128:tb_ * 128 + 128],
                                g[:, ntg_lo + tb_ - hi_base, :], start=False,
                                stop=(t == n_hi - 1))
                        if relu:
                            hw_ = work.tile([128, 64], F32, tag="hw")
                            nc.vector.scalar_tensor_tensor(
                                out=hw_[:], in0=ps[:], scalar=dinvp[:, w:w + 1],
                                in1=bias_t[:], op0=AG.mult, op1=AG.add)
                            hwb = work.tile([128, 64], BF, tag="hwb")
                            nc.vector.tensor_scalar(
                                out=hwb[:], in0=hw_[:], scalar1=0.0,
                                scalar2=None, op0=AG.max)
                            pt2 = psR.tile([128, 128], BF, tag="ptr")
                            nc.tensor.transpose(pt2[:64, :nw], hwb[:nw, :],
                                                id128[:nw, :nw])
                            j, r0 = (w * 128) // 512, (w * 128) % 512
                            nc.scalar.copy(hT_next[j][:, r0:r0 + nw],
                                           pt2[:64, :nw])
                        else:
                            # stage 4 windows into a chunk tile, flush as one
                            # strided DMA (fewer HWDGE ops)
                            j = (w * 128) // 512
                            if w % 4 == 0:
                                ost[0] = work.tile([128, 256], F32,
                                                   name="ost", tag="ost")
                            nc.vector.scalar_tensor_tensor(
                                out=ost[0][:, (w % 4) * 64:(w % 4) * 64 + 64],
                                in0=ps[:], scalar=dinvp[:, w:w + 1],
                                in1=bias_t[:], op0=AG.mult, op1=AG.add)
                            if w == W - 1 or w % 4 == 3:
                                c0 = 512 * j
                                cn = min(512, NLOC - c0)
                                if cn == 512:
                                    nc.sync.dma_start(
                                        out_d[c0:c0 + cn, :].rearrange(
                                            "(w p) e -> p w e", p=128),
                                        ost[0][:, :256])
                                else:
                                    for w_ in range(4 * j, W):
                                        nw_ = min(128, NLOC - w_ * 128)
                                        nc.sync.dma_start(
                                            out_d[w_ * 128:w_ * 128 + nw_, :],
                                            ost[0][:nw_, (w_ % 4) * 64:
                                                   (w_ % 4) * 64 + 64])
                    if nxt is not None:
                        done = g0 + gn  # windows completed
                        while (emitted < NCH
                               and (emitted + 1) * 512 <= done * 128):
                            nxt[0](emitted)
                            emitted += 1
                if nxt is not None:
                    while emitted < NCH:
                        nxt[0](emitted)
                        emitted += 1
                    nxt[1]()

            # hT3 aliases hT2: transform-2 chunk j (the only hT2[j] reader)
            # strictly precedes any L2-agg epilogue write (L2 agg waits on the
            # complete layer-2 table), so reuse is hazard-free.
            hT2 = [hTp.tile([64, 512], BF, name=f"hT2_{j}", tag=f"hT2_{j}")
                   for j in range(NCH)]
            hT3 = hT2
            PH = int(os.environ.get("K_PHASES", "9"))
            st0, em0, fin0 = make_transform(0, None, 128, W1t)
            st1, em1, fin1 = make_transform(1, hT2, 64, W2t)
            st2, em2, fin2 = make_transform(2, hT3, 64, W34t)
            if PH >= 1:
                for j in range(NCH):
                    em0(j)
                fin0()
            if PH >= 3:
                aggregate(0, st0, b1t, True, hT2,
                          nxt=(em1, fin1) if PH >= 4 else None)
            if PH >= 6:
                aggregate(1, st1, b2t, True, hT3,
                          nxt=(em2, fin2) if PH >= 7 else None)
            if PH >= 9:
                aggregate(2, st2, b34t, False, None)

    nc.compile()
    return nc


def _run(inputs, N, E):
    NLOC = N // N_CORES
    W = (NLOC + 127) // 128
    GRP = int(os.environ.get("K_GRP", "7"))

    x = np.asarray(inputs["x"], np.float32)
    ei = np.asarray(inputs["edge_index"], np.int64)
    W1 = np.asarray(inputs["W1"], np.float32)
    b1 = np.asarray(inputs["b1"], np.float32)
    W2 = np.asarray(inputs["W2"], np.float32)
    b2 = np.asarray(inputs["b2"], np.float32)
    Wmu = np.asarray(inputs["Wmu"], np.float32)
    bmu = np.asarray(inputs["bmu"], np.float32)
    Wls = np.asarray(inputs["Wls"], np.float32)
    bls = np.asarray(inputs["bls"], np.float32)

    meta, lpos, deg, idx_tiles, S_arr = _prep(x, ei, N, NLOC, W, GRP)

    key = (N, NLOC, W, meta["T_tot"], os.environ.get("K_PHASES", "9"))
    if key not in _cache:
        _cache[key] = _build(meta, solo=False)
    nc = _cache[key]

    W34 = np.concatenate([Wmu, Wls], axis=1)
    b34 = np.concatenate([bmu, bls])
    id64 = np.eye(64, dtype=np.float32).astype(BF16)
    id128 = np.eye(128, dtype=np.float32).astype(BF16)
    b1b = np.ascontiguousarray(np.tile(b1, (128, 1)))
    b2b = np.ascontiguousarray(np.tile(b2, (128, 1)))
    b34b = np.ascontiguousarray(np.tile(b34, (128, 1)))

    in_maps = []
    for c in range(N_CORES):
        lc = lpos[c * NLOC:(c + 1) * NLOC]
        degp = np.ones(W * 128, np.float32)
        degp[lc] = deg[c * NLOC:(c + 1) * NLOC]
        xp = np.empty((NLOC, x.shape[1]), np.float32)
        xp[lc] = x[c * NLOC:(c + 1) * NLOC]
        in_maps.append({
            "xT": np.ascontiguousarray(xp.T).astype(BF16),
            "idxs": idx_tiles[c],
            "S": np.ascontiguousarray(S_arr[c]),
            "degp": np.ascontiguousarray(degp.reshape(W, 128).T),
            "W1": W1.astype(BF16), "W2": W2.astype(BF16),
            "W34": W34.astype(BF16),
            "b1b": b1b, "b2b": b2b, "b34b": b34b,
            "id64": id64, "id128": id128,
        })

    _last["nc"] = nc
    _last["in_maps"] = in_maps
    res = run_bass_kernel_spmd(nc, in_maps, core_ids=list(range(N_CORES)))
    out = np.empty((N, 64), np.float32)
    for c in range(N_CORES):
        lc = lpos[c * NLOC:(c + 1) * NLOC]
        out[c * NLOC:(c + 1) * NLOC] = res.results[c]["out34"][lc]
    return out[:, :32].copy(), out[:, 32:].copy()


def kernel(**inputs):
    x = np.asarray(inputs["x"])
    ei = np.asarray(inputs["edge_index"])
    return _run(inputs, x.shape[0], ei.shape[1])


# revision 21
# speedup vs baseline: 1.0722x; 1.0722x over previous
"""3-layer GCN encoder (GCNConv x4, layers 3+4 fused) on 8 Trainium2 NeuronCores.

Strategy (graph/data parallel, per the edge-cut sharding hint):
  - Nodes partitioned contiguously across 8 cores (NLOC = N/8), degree-balanced
    window assignment (128 dst nodes per window) within each core.
  - Per layer: local transform T = h @ W on PE (bf16), rows scaled by dinv[src],
    written as a strided DRAM table (256B row stride; fp8 payload for layers
    1-2, bf16 for the final fused layer), AllGather -> full table per core.
  - Aggregation: per window group, dma_gather fetches each in-edge's source row
    (64B fp8 / 128B bf16 payload per descriptor -- raw-constructed gather
    instructions allow payload < 256B while the row stride stays 256B).
    Segment-sum runs on PE: per 128-slot tile, one matmul with a
    host-precomputed one-hot selection matrix S (fp8, SBUF-resident, shared by
    all three layers; zero columns at pad slots). Self-loop contributions come
    from the local stage tile via an identity matmul that also initializes the
    PSUM accumulator.
  - Epilogue per window: DVE (psum * dinv[dst] + bias), ReLU+bf16 cast, PE
    transpose into the next layer's feature-major input; final layer DMAs
    [nw, 64] = [mu | logstd] f32 rows to DRAM.
  - Gather indices are int16: rows >= 32768 are fetched by a second gather with
    the table base offset by 32768 rows (edges pre-split into lo/hi groups).

Self-contained: only needs numpy/ml_dtypes/concourse (container-installed).
"""

import os
import sys

if "/opt/trn_rl_repo" not in sys.path:
    sys.path.insert(0, "/opt/trn_rl_repo")

import numpy as np
import ml_dtypes

import concourse.bass as bass
import concourse.bacc as bacc
import concourse.mybir as mybir
import concourse.tile as tile
import concourse.ap_utils as ap_utils
from concourse.bass_utils import run_bass_kernel_spmd

BF16 = ml_dtypes.bfloat16
FP8 = ml_dtypes.float8_e4m3fn
F32 = mybir.dt.float32
BF = mybir.dt.bfloat16
E4 = mybir.dt.float8e4
I16 = mybir.dt.int16

N_CORES = 8
CHUNK = 32768  # int16 index range per gather call

_cache = {}
_last = {}


def last_run(trace=False, **kw):
    """Re-run the last compiled kernel/in_maps (optionally with NTFF tracing)."""
    if "nc" not in _last:
        return None
    return run_bass_kernel_spmd(_last["nc"], _last["in_maps"],
                                core_ids=list(range(N_CORES)), trace=trace, **kw)


def _balance(deg, N, NLOC, W):
    """Deal degree-sorted nodes round-robin into windows, per core."""
    lpos = np.empty(N, np.int64)
    caps = np.full(W, 128, np.int64)
    caps[W - 1] = NLOC - 128 * (W - 1)
    for c in range(N_CORES):
        dl = deg[c * NLOC:(c + 1) * NLOC]
        order_ = np.argsort(-dl, kind="stable")
        fill = np.zeros(W, np.int64)
        wi = 0
        pos = np.empty(NLOC, np.int64)
        for i in range(NLOC):
            while fill[wi % W] >= caps[wi % W]:
                wi += 1
            ww = wi % W
            pos[order_[i]] = ww * 128 + fill[ww]
            fill[ww] += 1
            wi += 1
        lpos[c * NLOC:(c + 1) * NLOC] = pos
    return lpos


def _prep(x, ei, N, NLOC, W, GRP):
    """Host-side edge bookkeeping: slot layout, gather indices, one-hot S."""
    src = ei[0].astype(np.int64)
    dst = ei[1].astype(np.int64)
    E = src.shape[0]
    deg = np.bincount(dst, minlength=N).astype(np.float32) + 1.0  # + self loop

    lpos = _balance(deg, N, NLOC, W)
    grow = (np.arange(N, dtype=np.int64) // NLOC) * NLOC + lpos

    srow = grow[src]
    c = dst // NLOC
    dloc = lpos[dst]
    w = dloc >> 7
    p = dloc & 127
    hi = (srow >= CHUNK).astype(np.int64)
    rel = (srow - hi * CHUNK).astype(np.int16)

    key = (c * W + w) * 2 + hi
    order = np.argsort(key, kind="stable")
    cnt = np.bincount(key, minlength=N_CORES * W * 2).reshape(N_CORES, W, 2)
    nt = (cnt.max(axis=0) + 127) // 128  # [W, 2] tiles per (window, chunk)

    # group/tile layout: per group of GRP windows, [all lo tiles | all hi tiles]
    tile_base = np.zeros((W, 2), np.int64)
    groups = []
    tb = 0
    for g0 in range(0, W, GRP):
        gn = min(GRP, W - g0)
        lo_base = tb
        for w_ in range(g0, g0 + gn):
            tile_base[w_, 0] = tb
            tb += nt[w_, 0]
        hi_base = tb
        for w_ in range(g0, g0 + gn):
            tile_base[w_, 1] = tb
            tb += nt[w_, 1]
        groups.append((g0, gn, int(lo_base), int(hi_base),
                       int(hi_base - lo_base), int(tb - hi_base)))
    T_tot = int(tb)
    P_tot = 128 * T_tot

    gstart = np.concatenate([[0], np.cumsum(cnt.reshape(-1))])
    pos_in_grp = np.arange(E) - gstart[key[order]]
    slot = tile_base[w[order], hi[order]] * 128 + pos_in_grp

    idx_arr = np.zeros((N_CORES, P_tot), np.int16)
    idx_arr[c[order], slot] = rel[order]
    S_arr = np.zeros((N_CORES, 128, P_tot), np.uint8)
    one = np.float32(1.0).astype(FP8).view(np.uint8)
    S_arr[c[order], slot % 128, (slot // 128) * 128 + p[order]] = one

    idx_tiles = []
    for cc in range(N_CORES):
        idx16 = np.ascontiguousarray(idx_arr[cc].reshape(P_tot // 16, 16).T)
        idx_tiles.append(np.ascontiguousarray(np.tile(idx16, (8, 1))))

    meta = {
        "N": N, "NLOC": NLOC, "W": W, "GRP": GRP,
        "nt": nt.tolist(), "tile_base": tile_base.tolist(),
        "groups": groups, "T_tot": T_tot, "P_tot": P_tot,
    }
    return meta, lpos, deg, idx_tiles, S_arr.view(FP8)


def _gather_raw(gp, out_ap, in_ap, idxs_ap, num_idxs, elem_size, elem_step):
    """dma_gather without the elem_size_bytes % 256 == 0 restriction.

    in_ap: DRAM [rows (stride elem_step elems), elem_size elems]
    out_ap: SBUF [128, num_idxs//128, elem_size]; idxs_ap: [128, num_idxs//16].
    """
    assert idxs_ap.dtype == mybir.dt.int16
    assert in_ap.dtype == out_ap.dtype
    assert in_ap.ap[0][0] == elem_step
    assert in_ap.ap[-1][1] == elem_size
    assert num_idxs % 128 == 0
    assert out_ap.ap[0][1] * out_ap.ap[1][1] == num_idxs
    assert ap_utils.ap_is_contiguous(out_ap.ap[1:])
    assert ap_utils.ap_is_contiguous(idxs_ap.ap[1:])
    stride_bytes = elem_step * mybir.dt.size(in_ap.dtype)
    stride_bytes_256 = stride_bytes // 256
    assert stride_bytes_256 * 256 == stride_bytes and 0 < stride_bytes_256 < 256
    return gp.add_instruction(
        mybir.InstDMAGatherAnt(
            name=gp.bass.get_next_instruction_name(),
            ins=[
                *gp.lower_ap_dma(in_ap, for_custom_bir_dma=True),
                gp.lower_ap(idxs_ap),
                gp.lower_val_access(gp.to_reg(num_idxs)),
            ],
            outs=[gp.lower_ap(out_ap)],
            transpose=False,
            num_idxs=num_idxs,
            elem_size=elem_size,
            stride_bytes_256=stride_bytes_256,
            gen_mode=0,
            single_packet=False,
            queue_num=0,
            sbuf_tokens_per_rank=0,
            sbuf_free_dim_per_rank=0,
            sbuf_free_dim_pad_per_rank=0,
            sbuf_byte_offset=0,
        )
    )


def _build(meta, solo):
    """Build the SPMD Bass program (solo=True: 1 core, no collectives)."""
    N, NLOC, W = meta["N"], meta["NLOC"], meta["W"]
    nt = meta["nt"]
    tile_base = meta["tile_base"]
    groups = meta["groups"]
    P_tot = meta["P_tot"]
    NTG_MAX = max(g[4] + g[5] for g in groups)
    AG = mybir.AluOpType
    RG = [list(range(N_CORES))]
    AF = mybir.ActivationFunctionType

    nc = bacc.Bacc("TRN2", target_bir_lowering=False, debug=False,
                   num_devices=1 if solo else N_CORES)

    xT_d = nc.dram_tensor("xT", (128, NLOC), BF, kind="ExternalInput")
    idxs_d = nc.dram_tensor("idxs", (128, P_tot // 16), I16,
                            kind="ExternalInput")
    S_d = nc.dram_tensor("S", (128, P_tot), E4, kind="ExternalInput")
    degp_d = nc.dram_tensor("degp", (128, W), F32, kind="ExternalInput")
    W1_d = nc.dram_tensor("W1", (128, 64), BF, kind="ExternalInput")
    W2_d = nc.dram_tensor("W2", (64, 64), BF, kind="ExternalInput")
    W34_d = nc.dram_tensor("W34", (64, 64), BF, kind="ExternalInput")
    b1_d = nc.dram_tensor("b1b", (128, 64), F32, kind="ExternalInput")
    b2_d = nc.dram_tensor("b2b", (128, 64), F32, kind="ExternalInput")
    b34_d = nc.dram_tensor("b34b", (128, 64), F32, kind="ExternalInput")
    id64_d = nc.dram_tensor("id64", (64, 64), BF, kind="ExternalInput")
    id128_d = nc.dram_tensor("id128", (128, 128), BF, kind="ExternalInput")
    out_d = nc.dram_tensor("out34", (NLOC, 64), F32, kind="ExternalOutput")

    ldt = [E4, E4, BF]                 # per-layer message dtype
    lcols = [256, 256, 128]            # table cols (256B row stride)
    tabl = [nc.dram_tensor(f"tabl{l}", (NLOC, lcols[l]), ldt[l],
                           kind="Internal") for l in range(3)]
    tabf = [nc.dram_tensor(f"tabf{l}", (N, lcols[l]), ldt[l], kind="Internal",
                           addr_space="Shared") for l in range(3)]

    with tile.TileContext(nc) as tc:
        with (
            tc.tile_pool(name="const", bufs=1) as const,
            tc.tile_pool(name="ttp", bufs=2) as ttp,
            tc.tile_pool(name="stg", bufs=2) as stg,
            tc.tile_pool(name="hTp", bufs=1) as hTp,
            tc.tile_pool(name="work", bufs=int(os.environ.get("K_WK", "8"))) as work,
            tc.tile_pool(name="gp", bufs=2) as gp,
            tc.tile_pool(name="psT", bufs=2, space="PSUM") as psT,
            tc.tile_pool(name="psR", bufs=2, space="PSUM") as psR,
            tc.tile_pool(name="psA", bufs=int(os.environ.get("K_PSA", "4")),
                         space="PSUM") as psA,
        ):
            def cload(dram, shape, dt, tag):
                t = const.tile(shape, dt, tag=tag)
                nc.sync.dma_start(t[:], dram[:])
                return t

            W1t = cload(W1_d, [128, 64], BF, "W1t")
            W2t = cload(W2_d, [64, 64], BF, "W2t")
            W34t = cload(W34_d, [64, 64], BF, "W34t")
            b1t = cload(b1_d, [128, 64], F32, "b1t")
            b2t = cload(b2_d, [128, 64], F32, "b2t")
            b34t = cload(b34_d, [128, 64], F32, "b34t")
            id64 = cload(id64_d, [64, 64], BF, "id64")
            id128 = cload(id128_d, [128, 128], BF, "id128")

            degp = cload(degp_d, [128, W], F32, "degp")
            sqp = const.tile([128, W], F32, tag="sqp")
            nc.scalar.activation(sqp[:], degp[:], AF.Sqrt)
            dinvp = const.tile([128, W], F32, tag="dinvp")
            nc.vector.reciprocal(dinvp[:], sqp[:])

            # S/idxs are loaded lazily, paced one aggregation group ahead so
            # the 38us S stream never crowds gathers off the DMA engines.
            idxs = const.tile([128, P_tot // 16], I16, tag="idxs")
            S = const.tile([128, P_tot], E4, tag="S")
            SCH = 1680  # S columns per load piece
            sload = [0]

            def load_S_through(tiles):
                """Issue S/idxs pieces covering slot-tiles [0, tiles)."""
                need = min(tiles * 128, P_tot)
                while sload[0] < need:
                    c0 = sload[0]
                    cn = min(SCH, P_tot - c0)
                    nc.sync.dma_start(S[:, c0:c0 + cn], S_d[:, c0:c0 + cn])
                    nc.sync.dma_start(idxs[:, c0 // 16:(c0 + cn) // 16],
                                      idxs_d[:, c0 // 16:(c0 + cn) // 16])
                    sload[0] += cn

            NCH = (NLOC + 511) // 512

            def make_transform(l, hT, K, Wt):
                """Returns (stage, emit_chunk, finish). emit_chunk(j): compute
                T = h @ W for nodes [512j, 512j+cn), dinv[src]-scaled and
                transposed into stage, table rows written strided to DRAM."""
                stage = stg.tile([128, W * 64], ldt[l], tag="stage")

                def emit(j):
                    c0 = 512 * j
                    cn = min(512, NLOC - c0)
                    if hT is None:
                        xc = work.tile([128, 512], BF, tag="xc")
                        nc.sync.dma_start(xc[:, :cn], xT_d[:, c0:c0 + cn])
                        rhs = xc[:K, :cn]
                    else:
                        rhs = hT[j][:K, :cn]
                    ps = psT.tile([64, 512], F32, tag="psT")
                    nc.tensor.matmul(ps[:, :cn], Wt[:K, :], rhs,
                                     start=True, stop=True)
                    TTc = ttp.tile([64, 512], BF, tag="TTc")
                    nc.vector.tensor_copy(TTc[:, :cn], ps[:, :cn])
                    w0, w1 = c0 // 128, min((c0 + cn + 127) // 128, W)
                    for w in range(w0, w1):
                        off = w * 128 - c0
                        nw = min(128, NLOC - w * 128)
                        ptt = psR.tile([128, 128], BF, tag="ptr")
                        nc.tensor.transpose(ptt[:nw, :64], TTc[:, off:off + nw],
                                            id64[:, :])
                        nc.scalar.activation(
                            stage[:nw, 64 * w:64 * w + 64], ptt[:nw, :64],
                            AF.Copy, scale=dinvp[:nw, w:w + 1])
                        if nw < 128:
                            nc.vector.memset(stage[nw:, 64 * w:64 * w + 64], 0.0)
                    if cn == 512:
                        nc.sync.dma_start(
                            tabl[l][c0:c0 + cn, 0:64].rearrange(
                                "(w p) e -> p w e", p=128),
                            stage[:, 64 * w0:64 * w1])
                    else:
                        for w in range(w0, w1):
                            nw = min(128, NLOC - w * 128)
                            nc.sync.dma_start(
                                tabl[l][w * 128:w * 128 + nw, 0:64],
                                stage[:nw, 64 * w:64 * w + 64])

                def finish():
                    if not solo:
                        nc.gpsimd.collective_compute(
                            "AllGather", AG.bypass, replica_groups=RG,
                            ins=[tabl[l][:].opt()], outs=[tabf[l][:].opt()])

                return stage, emit, finish

            def aggregate(l, stage, bias_t, relu, hT_next, nxt=None):
                """nxt = (emit, finish) of the next layer's transform; chunks
                are emitted as soon as their source windows are done."""
                in_lo = tabf[l][0:CHUNK, 0:64]
                in_hi = tabf[l][CHUNK:N, 0:64]
                emitted = 0
                ost = [None]
                if l == 0:
                    load_S_through(groups[0][3] + groups[0][5])
                for gi, (g0, gn, lo_base, hi_base, ntg_lo, ntg_hi) \
                        in enumerate(groups):
                    g = gp.tile([128, NTG_MAX, 64], ldt[l], tag="g")
                    if l == 0 and gi + 1 < len(groups):
                        nxt_g = groups[gi + 1]
                        load_S_through(nxt_g[3] + nxt_g[5])
                    if ntg_lo:
                        _gather_raw(
                            nc.gpsimd, g[:, :ntg_lo, :], in_lo,
                            idxs[:, lo_base * 8:(lo_base + ntg_lo) * 8],
                            ntg_lo * 128, 64, lcols[l])
                    if ntg_hi:
                        _gather_raw(
                            nc.gpsimd, g[:, ntg_lo:ntg_lo + ntg_hi, :], in_hi,
                            idxs[:, hi_base * 8:(hi_base + ntg_hi) * 8],
                            ntg_hi * 128, 64, lcols[l])
                    for w in range(g0, g0 + gn):
                        nw = min(128, NLOC - w * 128)
                        n_lo, n_hi = nt[w][0], nt[w][1]
                        ps = psA.tile([128, 64], F32, tag="psA")
                        nc.tensor.matmul(ps[:], id128[:, :],
                                         stage[:, 64 * w:64 * w + 64],
                                         start=True, stop=(n_lo + n_hi == 0))
                        for t in range(n_lo):
                            tb_ = tile_base[w][0] + t
                            nc.tensor.matmul(
                                ps[:], S[:, tb_ * 128:tb_ * 128 + 128],
                                g[:, tb_ - lo_base, :], start=False,
                                stop=(n_hi == 0 and t == n_lo - 1))
                        for t in range(n_hi):
                            tb_ = tile_base[w][1] + t
                            nc.tensor.matmul(
                                ps[:], S[:, tb_ * 128:tb_ * 128 + 128],
                                g[:, ntg_lo + tb_ - hi_base, :], start=False,
                                stop=(t == n_hi - 1))
                        if relu:
                            hw_ = work.tile([128, 64], F32, tag="hw")
                            nc.vector.scalar_tensor_tensor(
                                out=hw_[:], in0=ps[:], scalar=dinvp[:, w:w + 1],
                                in1=bias_t[:], op0=AG.mult, op1=AG.add)
                            hwb = work.tile([128, 64], BF, tag="hwb")
                            nc.vector.tensor_scalar(
                                out=hwb[:], in0=hw_[:], scalar1=0.0,
                                scalar2=None, op0=AG.max)
                            pt2 = psR.tile([128, 128], BF, tag="ptr")
                            nc.tensor.transpose(pt2[:64, :nw], hwb[:nw, :],
                                                id128[:nw, :nw])
                            j, r0 = (w * 128) // 512, (w * 128) % 512
                            nc.scalar.copy(hT_next[j][:, r0:r0 + nw],
                                           pt2[:64, :nw])
                        else:
                            # stage 4 windows into a chunk tile, flush as one
                            # strided DMA (fewer HWDGE ops)
                            j = (w * 128) // 512
                            if w % 4 == 0:
                                ost[0] = work.tile([128, 256], F32,
                                                   name="ost", tag="ost")
                            nc.vector.scalar_tensor_tensor(
                                out=ost[0][:, (w % 4) * 64:(w % 4) * 64 + 64],
                                in0=ps[:], scalar=dinvp[:, w:w + 1],
                                in1=bias_t[:], op0=AG.mult, op1=AG.add)
                            if w == W - 1 or w % 4 == 3:
                                c0 = 512 * j
                                cn = min(512, NLOC - c0)
                                if cn == 512:
                                    nc.sync.dma_start(
                                        out_d[c0:c0 + cn, :].rearrange(
                                            "(w p) e -> p w e", p=128),
                                        ost[0][:, :256])
                                else:
                                    for w_ in range(4 * j, W):
                                        nw_ = min(128, NLOC - w_ * 128)
                                        nc.sync.dma_start(
                                            out_d[w_ * 128:w_ * 128 + nw_, :],
                                            ost[0][:nw_, (w_ % 4) * 64:
                                                   (w_ % 4) * 64 + 64])
                    if nxt is not None:
                        done = g0 + gn  # windows completed
                        while (emitted < NCH
                               and (emitted + 1) * 512 <= done * 128):
                            nxt[0](emitted)
                            emitted += 1
                if nxt is not None:
                    while emitted < NCH:
                        nxt[0](emitted)
                        emitted += 1
                    nxt[1]()

            # hT3 aliases hT2: transform-2 chunk j (the only hT2[j] reader)
            # strictly precedes any L2-agg epilogue write (L2 agg waits on the
            # complete layer-2 table), so reuse is hazard-free.
            hT2 = [hTp.tile([64, 512], BF, name=f"hT2_{j}", tag=f"hT2_{j}")
                   for j in range(NCH)]
            hT3 = hT2
            PH = int(os.environ.get("K_PHASES", "9"))
            st0, em0, fin0 = make_transform(0, None, 128, W1t)
            st1, em1, fin1 = make_transform(1, hT2, 64, W2t)
            st2, em2, fin2 = make_transform(2, hT3, 64, W34t)
            if PH >= 1:
                for j in range(NCH):
                    em0(j)
                fin0()
            if PH >= 3:
                aggregate(0, st0, b1t, True, hT2,
                          nxt=(em1, fin1) if PH >= 4 else None)
            if PH >= 6:
                aggregate(1, st1, b2t, True, hT3,
                          nxt=(em2, fin2) if PH >= 7 else None)
            if PH >= 9:
                aggregate(2, st2, b34t, False, None)

    nc.compile()
    return nc


def _run(inputs, N, E):
    NLOC = N // N_CORES
    W = (NLOC + 127) // 128
    GRP = int(os.environ.get("K_GRP", "7"))

    x = np.asarray(inputs["x"], np.float32)
    ei = np.asarray(inputs["edge_index"], np.int64)
    W1 = np.asarray(inputs["W1"], np.float32)
    b1 = np.asarray(inputs["b1"], np.float32)
    W2 = np.asarray(inputs["W2"], np.float32)
    b2 = np.asarray(inputs["b2"], np.float32)
    Wmu = np.asarray(inputs["Wmu"], np.float32)
    bmu = np.asarray(inputs["bmu"], np.float32)
    Wls = np.asarray(inputs["Wls"], np.float32)
    bls = np.asarray(inputs["bls"], np.float32)

    meta, lpos, deg, idx_tiles, S_arr = _prep(x, ei, N, NLOC, W, GRP)

    key = (N, NLOC, W, meta["T_tot"], os.environ.get("K_PHASES", "9"))
    if key not in _cache:
        _cache[key] = _build(meta, solo=False)
    nc = _cache[key]

    W34 = np.concatenate([Wmu, Wls], axis=1)
    b34 = np.concatenate([bmu, bls])
    id64 = np.eye(64, dtype=np.float32).astype(BF16)
    id128 = np.eye(128, dtype=np.float32).astype(BF16)
    b1b = np.ascontiguousarray(np.tile(b1, (128, 1)))
    b2b = np.ascontiguousarray(np.tile(b2, (128, 1)))
    b34b = np.ascontiguousarray(np.tile(b34, (128, 1)))

    in_maps = []
    for c in range(N_CORES):
        lc = lpos[c * NLOC:(c + 1) * NLOC]
        degp = np.ones(W * 128, np.float32)
        degp[lc] = deg[c * NLOC:(c + 1) * NLOC]
        xp = np.empty((NLOC, x.shape[1]), np.float32)
        xp[lc] = x[c * NLOC:(c + 1) * NLOC]
        in_maps.append({
            "xT": np.ascontiguousarray(xp.T).astype(BF16),
            "idxs": idx_tiles[c],
            "S": np.ascontiguousarray(S_arr[c]),
            "degp": np.ascontiguousarray(degp.reshape(W, 128).T),
            "W1": W1.astype(BF16), "W2": W2.astype(BF16),
            "W34": W34.astype(BF16),
            "b1b": b1b, "b2b": b2b, "b34b": b34b,
            "id64": id64, "id128": id128,
        })

    _last["nc"] = nc
    _last["in_maps"] = in_maps
    res = run_bass_kernel_spmd(nc, in_maps, core_ids=list(range(N_CORES)))
    out = np.empty((N, 64), np.float32)
    for c in range(N_CORES):
        lc = lpos[c * NLOC:(c + 1) * NLOC]
        out[c * NLOC:(c + 1) * NLOC] = res.results[c]["out34"][lc]
    return out[:, :32].copy(), out[:, 32:].copy()


def kernel(**inputs):
    x = np.asarray(inputs["x"])
    ei = np.asarray(inputs["edge_index"])
    return _run(inputs, x.shape[0], ei.shape[1])


# revision 23
# speedup vs baseline: 1.1961x; 1.1156x over previous
"""3-layer GCN encoder (GCNConv x4, layers 3+4 fused) on 8 Trainium2 NeuronCores.

Strategy (graph/data parallel, per the edge-cut sharding hint):
  - Nodes partitioned contiguously across 8 cores (NLOC = N/8), degree-balanced
    window assignment (128 dst nodes per window) within each core.
  - Per layer: local transform T = h @ W on PE (bf16), rows scaled by dinv[src],
    written as a strided DRAM table (256B row stride; fp8 payload for layers
    1-2, bf16 for the final fused layer), AllGather -> full table per core.
  - Aggregation: per window group, dma_gather fetches each in-edge's source row
    (64B fp8 / 128B bf16 payload per descriptor -- raw-constructed gather
    instructions allow payload < 256B while the row stride stays 256B).
    Segment-sum runs on PE: per 128-slot tile, one matmul with a
    host-precomputed one-hot selection matrix S (fp8, SBUF-resident, shared by
    all three layers; zero columns at pad slots). Self-loop contributions come
    from the local stage tile via an identity matmul that also initializes the
    PSUM accumulator.
  - Epilogue per window: DVE (psum * dinv[dst] + bias), ReLU+bf16 cast, PE
    transpose into the next layer's feature-major input; final layer DMAs
    [nw, 64] = [mu | logstd] f32 rows to DRAM.
  - Gather indices are int16: rows >= 32768 are fetched by a second gather with
    the table base offset by 32768 rows (edges pre-split into lo/hi groups).

Self-contained: only needs numpy/ml_dtypes/concourse (container-installed).
"""

import os
import sys

if "/opt/trn_rl_repo" not in sys.path:
    sys.path.insert(0, "/opt/trn_rl_repo")

import numpy as np
import ml_dtypes

import concourse.bass as bass
import concourse.bacc as bacc
import concourse.mybir as mybir
import concourse.tile as tile
import concourse.ap_utils as ap_utils
from concourse.bass_utils import run_bass_kernel_spmd

BF16 = ml_dtypes.bfloat16
FP8 = ml_dtypes.float8_e4m3fn
F32 = mybir.dt.float32
BF = mybir.dt.bfloat16
E4 = mybir.dt.float8e4
I16 = mybir.dt.int16

N_CORES = 8
CHUNK = 32768  # int16 index range per gather call

_cache = {}
_last = {}


def last_run(trace=False, **kw):
    """Re-run the last compiled kernel/in_maps (optionally with NTFF tracing)."""
    if "nc" not in _last:
        return None
    return run_bass_kernel_spmd(_last["nc"], _last["in_maps"],
                                core_ids=list(range(N_CORES)), trace=trace, **kw)


def _balance(deg, N, NLOC, W):
    """Deal degree-sorted nodes round-robin into windows, per core."""
    lpos = np.empty(N, np.int64)
    caps = np.full(W, 128, np.int64)
    caps[W - 1] = NLOC - 128 * (W - 1)
    for c in range(N_CORES):
        dl = deg[c * NLOC:(c + 1) * NLOC]
        order_ = np.argsort(-dl, kind="stable")
        fill = np.zeros(W, np.int64)
        wi = 0
        pos = np.empty(NLOC, np.int64)
        for i in range(NLOC):
            while fill[wi % W] >= caps[wi % W]:
                wi += 1
            ww = wi % W
            pos[order_[i]] = ww * 128 + fill[ww]
            fill[ww] += 1
            wi += 1
        lpos[c * NLOC:(c + 1) * NLOC] = pos
    return lpos


def _prep(x, ei, N, NLOC, W, GRP):
    """Host-side edge bookkeeping: slot layout, gather indices, one-hot S."""
    src = ei[0].astype(np.int64)
    dst = ei[1].astype(np.int64)
    E = src.shape[0]
    deg = np.bincount(dst, minlength=N).astype(np.float32) + 1.0  # + self loop

    lpos = _balance(deg, N, NLOC, W)
    grow = (np.arange(N, dtype=np.int64) // NLOC) * NLOC + lpos

    srow = grow[src]
    c = dst // NLOC
    dloc = lpos[dst]
    w = dloc >> 7
    p = dloc & 127
    hi = (srow >= CHUNK).astype(np.int64)
    rel = (srow - hi * CHUNK).astype(np.int16)

    key = (c * W + w) * 2 + hi
    order = np.argsort(key, kind="stable")
    cnt = np.bincount(key, minlength=N_CORES * W * 2).reshape(N_CORES, W, 2)
    nt = (cnt.max(axis=0) + 127) // 128  # [W, 2] tiles per (window, chunk)

    # group/tile layout: per group of GRP windows, [all lo tiles | all hi tiles]
    tile_base = np.zeros((W, 2), np.int64)
    groups = []
    tb = 0
    for g0 in range(0, W, GRP):
        gn = min(GRP, W - g0)
        lo_base = tb
        for w_ in range(g0, g0 + gn):
            tile_base[w_, 0] = tb
            tb += nt[w_, 0]
        hi_base = tb
        for w_ in range(g0, g0 + gn):
            tile_base[w_, 1] = tb
            tb += nt[w_, 1]
        groups.append((g0, gn, int(lo_base), int(hi_base),
                       int(hi_base - lo_base), int(tb - hi_base)))
    T_tot = int(tb)
    P_tot = 128 * T_tot

    gstart = np.concatenate([[0], np.cumsum(cnt.reshape(-1))])
    pos_in_grp = np.arange(E) - gstart[key[order]]
    slot = tile_base[w[order], hi[order]] * 128 + pos_in_grp

    idx_arr = np.zeros((N_CORES, P_tot), np.int16)
    idx_arr[c[order], slot] = rel[order]
    S_arr = np.zeros((N_CORES, 128, P_tot), np.uint8)
    one = np.float32(1.0).astype(FP8).view(np.uint8)
    S_arr[c[order], slot % 128, (slot // 128) * 128 + p[order]] = one

    idx_tiles = []
    for cc in range(N_CORES):
        idx16 = np.ascontiguousarray(idx_arr[cc].reshape(P_tot // 16, 16).T)
        idx_tiles.append(np.ascontiguousarray(np.tile(idx16, (8, 1))))

    meta = {
        "N": N, "NLOC": NLOC, "W": W, "GRP": GRP,
        "nt": nt.tolist(), "tile_base": tile_base.tolist(),
        "groups": groups, "T_tot": T_tot, "P_tot": P_tot,
    }
    return meta, lpos, deg, idx_tiles, S_arr.view(FP8)


def _gather_raw(gp, out_ap, in_ap, idxs_ap, num_idxs, elem_size, elem_step):
    """dma_gather without the elem_size_bytes % 256 == 0 restriction.

    in_ap: DRAM [rows (stride elem_step elems), elem_size elems]
    out_ap: SBUF [128, num_idxs//128, elem_size]; idxs_ap: [128, num_idxs//16].
    """
    assert idxs_ap.dtype == mybir.dt.int16
    assert in_ap.dtype == out_ap.dtype
    assert in_ap.ap[0][0] == elem_step
    assert in_ap.ap[-1][1] == elem_size
    assert num_idxs % 128 == 0
    assert out_ap.ap[0][1] * out_ap.ap[1][1] == num_idxs
    assert ap_utils.ap_is_contiguous(out_ap.ap[1:])
    assert ap_utils.ap_is_contiguous(idxs_ap.ap[1:])
    stride_bytes = elem_step * mybir.dt.size(in_ap.dtype)
    stride_bytes_256 = stride_bytes // 256
    assert stride_bytes_256 * 256 == stride_bytes and 0 < stride_bytes_256 < 256
    return gp.add_instruction(
        mybir.InstDMAGatherAnt(
            name=gp.bass.get_next_instruction_name(),
            ins=[
                *gp.lower_ap_dma(in_ap, for_custom_bir_dma=True),
                gp.lower_ap(idxs_ap),
                gp.lower_val_access(gp.to_reg(num_idxs)),
            ],
            outs=[gp.lower_ap(out_ap)],
            transpose=False,
            num_idxs=num_idxs,
            elem_size=elem_size,
            stride_bytes_256=stride_bytes_256,
            gen_mode=0,
            single_packet=False,
            queue_num=0,
            sbuf_tokens_per_rank=0,
            sbuf_free_dim_per_rank=0,
            sbuf_free_dim_pad_per_rank=0,
            sbuf_byte_offset=0,
        )
    )


def _build(meta, solo):
    """Build the SPMD Bass program (solo=True: 1 core, no collectives)."""
    N, NLOC, W = meta["N"], meta["NLOC"], meta["W"]
    nt = meta["nt"]
    tile_base = meta["tile_base"]
    groups = meta["groups"]
    P_tot = meta["P_tot"]
    NTG_MAX = max(g[4] + g[5] for g in groups)
    AG = mybir.AluOpType
    RG = [list(range(N_CORES))]
    AF = mybir.ActivationFunctionType

    nc = bacc.Bacc("TRN2", target_bir_lowering=False, debug=False,
                   num_devices=1 if solo else N_CORES)

    xT_d = nc.dram_tensor("xT", (128, NLOC), BF, kind="ExternalInput")
    idxs_d = nc.dram_tensor("idxs", (128, P_tot // 16), I16,
                            kind="ExternalInput")
    S_d = nc.dram_tensor("S", (128, P_tot), E4, kind="ExternalInput")
    degp_d = nc.dram_tensor("degp", (128, W), F32, kind="ExternalInput")
    W1_d = nc.dram_tensor("W1", (128, 64), BF, kind="ExternalInput")
    W2_d = nc.dram_tensor("W2", (64, 64), BF, kind="ExternalInput")
    W34_d = nc.dram_tensor("W34", (64, 64), BF, kind="ExternalInput")
    b1_d = nc.dram_tensor("b1b", (128, 64), F32, kind="ExternalInput")
    b2_d = nc.dram_tensor("b2b", (128, 64), F32, kind="ExternalInput")
    b34_d = nc.dram_tensor("b34b", (128, 64), F32, kind="ExternalInput")
    id64_d = nc.dram_tensor("id64", (64, 64), BF, kind="ExternalInput")
    id128_d = nc.dram_tensor("id128", (128, 128), BF, kind="ExternalInput")
    out_d = nc.dram_tensor("out34", (NLOC, 64), F32, kind="ExternalOutput")

    ldt = [E4, E4, BF]                 # per-layer message dtype
    lcols = [256, 256, 128]            # table cols (256B row stride)
    tabl = [nc.dram_tensor(f"tabl{l}", (NLOC, lcols[l]), ldt[l],
                           kind="Internal") for l in range(3)]
    tabf = [nc.dram_tensor(f"tabf{l}", (N, lcols[l]), ldt[l], kind="Internal",
                           addr_space="Shared") for l in range(3)]

    with tile.TileContext(nc) as tc:
        with (
            tc.tile_pool(name="const", bufs=1) as const,
            tc.tile_pool(name="ttp", bufs=2) as ttp,
            tc.tile_pool(name="stg", bufs=2) as stg,
            tc.tile_pool(name="hTp", bufs=1) as hTp,
            tc.tile_pool(name="work", bufs=int(os.environ.get("K_WK", "8"))) as work,
            tc.tile_pool(name="gp", bufs=2) as gp,
            tc.tile_pool(name="psT", bufs=2, space="PSUM") as psT,
            tc.tile_pool(name="psR", bufs=2, space="PSUM") as psR,
            tc.tile_pool(name="psA", bufs=int(os.environ.get("K_PSA", "4")),
                         space="PSUM") as psA,
        ):
            def cload(dram, shape, dt, tag):
                t = const.tile(shape, dt, tag=tag)
                nc.sync.dma_start(t[:], dram[:])
                return t

            W1t = cload(W1_d, [128, 64], BF, "W1t")
            W2t = cload(W2_d, [64, 64], BF, "W2t")
            W34t = cload(W34_d, [64, 64], BF, "W34t")
            b1t = cload(b1_d, [128, 64], F32, "b1t")
            b2t = cload(b2_d, [128, 64], F32, "b2t")
            b34t = cload(b34_d, [128, 64], F32, "b34t")
            id64 = cload(id64_d, [64, 64], BF, "id64")
            id128 = cload(id128_d, [128, 128], BF, "id128")

            degp = cload(degp_d, [128, W], F32, "degp")
            sqp = const.tile([128, W], F32, tag="sqp")
            nc.scalar.activation(sqp[:], degp[:], AF.Sqrt)
            dinvp = const.tile([128, W], F32, tag="dinvp")
            nc.vector.reciprocal(dinvp[:], sqp[:])

            # big constant loads from the (early-idle) ACT queue; S in pieces
            # so small transform DMAs can interleave on the DMA engines
            idxs = const.tile([128, P_tot // 16], I16, tag="idxs")
            nc.scalar.dma_start(idxs[:], idxs_d[:])
            S = const.tile([128, P_tot], E4, tag="S")
            SCH = 1680
            for c0_ in range(0, P_tot, SCH):
                cn_ = min(SCH, P_tot - c0_)
                nc.scalar.dma_start(S[:, c0_:c0_ + cn_], S_d[:, c0_:c0_ + cn_])

            NCH = (NLOC + 511) // 512

            def make_transform(l, hT, K, Wt):
                """Returns (stage, emit_chunk, finish). emit_chunk(j): compute
                T = h @ W for nodes [512j, 512j+cn), dinv[src]-scaled and
                transposed into stage, table rows written strided to DRAM."""
                stage = stg.tile([128, W * 64], ldt[l], tag="stage")

                def emit(j):
                    c0 = 512 * j
                    cn = min(512, NLOC - c0)
                    if hT is None:
                        xc = work.tile([128, 512], BF, tag="xc")
                        nc.sync.dma_start(xc[:, :cn], xT_d[:, c0:c0 + cn])
                        hsrc = xc
                    else:
                        hsrc = hT[j]
                    w0, w1 = c0 // 128, min((c0 + cn + 127) // 128, W)
                    for w in range(w0, w1):
                        off = w * 128 - c0
                        nw = min(128, NLOC - w * 128)
                        # node-major rows directly: out[node, f] via
                        # lhsT = feature-major h slice, rhs = W
                        psW = psR.tile([128, 64], F32, tag="psW")
                        nc.tensor.matmul(psW[:nw, :], hsrc[:K, off:off + nw],
                                         Wt[:K, :], start=True, stop=True)
                        if w % 2 == 0:
                            nc.scalar.activation(
                                stage[:nw, 64 * w:64 * w + 64], psW[:nw, :],
                                AF.Copy, scale=dinvp[:nw, w:w + 1])
                        else:
                            nc.vector.tensor_scalar(
                                out=stage[:nw, 64 * w:64 * w + 64],
                                in0=psW[:nw, :], scalar1=dinvp[:nw, w:w + 1],
                                scalar2=None, op0=AG.mult)
                        if nw < 128:
                            nc.vector.memset(stage[nw:, 64 * w:64 * w + 64], 0.0)
                    if cn == 512:
                        nc.sync.dma_start(
                            tabl[l][c0:c0 + cn, 0:64].rearrange(
                                "(w p) e -> p w e", p=128),
                            stage[:, 64 * w0:64 * w1])
                    else:
                        for w in range(w0, w1):
                            nw = min(128, NLOC - w * 128)
                            nc.sync.dma_start(
                                tabl[l][w * 128:w * 128 + nw, 0:64],
                                stage[:nw, 64 * w:64 * w + 64])

                def finish():
                    if not solo:
                        nc.gpsimd.collective_compute(
                            "AllGather", AG.bypass, replica_groups=RG,
                            ins=[tabl[l][:].opt()], outs=[tabf[l][:].opt()])

                return stage, emit, finish

            def aggregate(l, stage, bias_t, relu, hT_next, nxt=None):
                """nxt = (emit, finish) of the next layer's transform; chunks
                are emitted as soon as their source windows are done."""
                in_lo = tabf[l][0:CHUNK, 0:64]
                in_hi = tabf[l][CHUNK:N, 0:64]
                emitted = 0
                ost = [None]
                for (g0, gn, lo_base, hi_base, ntg_lo, ntg_hi) in groups:
                    g = gp.tile([128, NTG_MAX, 64], ldt[l], tag="g")
                    if ntg_lo:
                        _gather_raw(
                            nc.gpsimd, g[:, :ntg_lo, :], in_lo,
                            idxs[:, lo_base * 8:(lo_base + ntg_lo) * 8],
                            ntg_lo * 128, 64, lcols[l])
                    if ntg_hi:
                        _gather_raw(
                            nc.gpsimd, g[:, ntg_lo:ntg_lo + ntg_hi, :], in_hi,
                            idxs[:, hi_base * 8:(hi_base + ntg_hi) * 8],
                            ntg_hi * 128, 64, lcols[l])
                    for w in range(g0, g0 + gn):
                        nw = min(128, NLOC - w * 128)
                        n_lo, n_hi = nt[w][0], nt[w][1]
                        ps = psA.tile([128, 64], F32, tag="psA")
                        nc.tensor.matmul(ps[:], id128[:, :],
                                         stage[:, 64 * w:64 * w + 64],
                                         start=True, stop=(n_lo + n_hi == 0))
                        for t in range(n_lo):
                            tb_ = tile_base[w][0] + t
                            nc.tensor.matmul(
                                ps[:], S[:, tb_ * 128:tb_ * 128 + 128],
                                g[:, tb_ - lo_base, :], start=False,
                                stop=(n_hi == 0 and t == n_lo - 1))
                        for t in range(n_hi):
                            tb_ = tile_base[w][1] + t
                            nc.tensor.matmul(
                                ps[:], S[:, tb_ * 128:tb_ * 128 + 128],
                                g[:, ntg_lo + tb_ - hi_base, :], start=False,
                                stop=(t == n_hi - 1))
                        if relu:
                            hw_ = work.tile([128, 64], F32, tag="hw")
                            nc.vector.scalar_tensor_tensor(
                                out=hw_[:], in0=ps[:], scalar=dinvp[:, w:w + 1],
                                in1=bias_t[:], op0=AG.mult, op1=AG.add)
                            hwb = work.tile([128, 64], BF, tag="hwb")
                            nc.vector.tensor_scalar(
                                out=hwb[:], in0=hw_[:], scalar1=0.0,
                                scalar2=None, op0=AG.max)
                            pt2 = psR.tile([128, 128], BF, tag="ptr")
                            nc.tensor.transpose(pt2[:64, :nw], hwb[:nw, :],
                                                id128[:nw, :nw])
                            j, r0 = (w * 128) // 512, (w * 128) % 512
                            nc.scalar.copy(hT_next[j][:, r0:r0 + nw],
                                           pt2[:64, :nw])
                        else:
                            # stage 4 windows into a chunk tile, flush as one
                            # strided DMA (fewer HWDGE ops)
                            j = (w * 128) // 512
                            if w % 4 == 0:
                                ost[0] = work.tile([128, 256], F32,
                                                   name="ost", tag="ost")
                            nc.vector.scalar_tensor_tensor(
                                out=ost[0][:, (w % 4) * 64:(w % 4) * 64 + 64],
                                in0=ps[:], scalar=dinvp[:, w:w + 1],
                                in1=bias_t[:], op0=AG.mult, op1=AG.add)
                            if w == W - 1 or w % 4 == 3:
                                c0 = 512 * j
                                cn = min(512, NLOC - c0)
                                if cn == 512:
                                    nc.sync.dma_start(
                                        out_d[c0:c0 + cn, :].rearrange(
                                            "(w p) e -> p w e", p=128),
                                        ost[0][:, :256])
                                else:
                                    for w_ in range(4 * j, W):
                                        nw_ = min(128, NLOC - w_ * 128)
                                        nc.sync.dma_start(
                                            out_d[w_ * 128:w_ * 128 + nw_, :],
                                            ost[0][:nw_, (w_ % 4) * 64:
                                                   (w_ % 4) * 64 + 64])
                    if nxt is not None:
                        done = g0 + gn  # windows completed
                        while (emitted < NCH
                               and (emitted + 1) * 512 <= done * 128):
                            nxt[0](emitted)
                            emitted += 1
                if nxt is not None:
                    while emitted < NCH:
                        nxt[0](emitted)
                        emitted += 1
                    nxt[1]()

            # hT3 aliases hT2: transform-2 chunk j (the only hT2[j] reader)
            # strictly precedes any L2-agg epilogue write (L2 agg waits on the
            # complete layer-2 table), so reuse is hazard-free.
            hT2 = [hTp.tile([64, 512], BF, name=f"hT2_{j}", tag=f"hT2_{j}")
                   for j in range(NCH)]
            hT3 = hT2
            PH = int(os.environ.get("K_PHASES", "9"))
            st0, em0, fin0 = make_transform(0, None, 128, W1t)
            st1, em1, fin1 = make_transform(1, hT2, 64, W2t)
            st2, em2, fin2 = make_transform(2, hT3, 64, W34t)
            if PH >= 1:
                for j in range(NCH):
                    em0(j)
                fin0()
            if PH >= 3:
                aggregate(0, st0, b1t, True, hT2,
                          nxt=(em1, fin1) if PH >= 4 else None)
            if PH >= 6:
                aggregate(1, st1, b2t, True, hT3,
                          nxt=(em2, fin2) if PH >= 7 else None)
            if PH >= 9:
                aggregate(2, st2, b34t, False, None)

    nc.compile()
    return nc


def _run(inputs, N, E):
    NLOC = N // N_CORES
    W = (NLOC + 127) // 128
    GRP = int(os.environ.get("K_GRP", "7"))

    x = np.asarray(inputs["x"], np.float32)
    ei = np.asarray(inputs["edge_index"], np.int64)
    W1 = np.asarray(inputs["W1"], np.float32)
    b1 = np.asarray(inputs["b1"], np.float32)
    W2 = np.asarray(inputs["W2"], np.float32)
    b2 = np.asarray(inputs["b2"], np.float32)
    Wmu = np.asarray(inputs["Wmu"], np.float32)
    bmu = np.asarray(inputs["bmu"], np.float32)
    Wls = np.asarray(inputs["Wls"], np.float32)
    bls = np.asarray(inputs["bls"], np.float32)

    meta, lpos, deg, idx_tiles, S_arr = _prep(x, ei, N, NLOC, W, GRP)

    key = (N, NLOC, W, meta["T_tot"], os.environ.get("K_PHASES", "9"))
    if key not in _cache:
        _cache[key] = _build(meta, solo=False)
    nc = _cache[key]

    W34 = np.concatenate([Wmu, Wls], axis=1)
    b34 = np.concatenate([bmu, bls])
    id64 = np.eye(64, dtype=np.float32).astype(BF16)
    id128 = np.eye(128, dtype=np.float32).astype(BF16)
    b1b = np.ascontiguousarray(np.tile(b1, (128, 1)))
    b2b = np.ascontiguousarray(np.tile(b2, (128, 1)))
    b34b = np.ascontiguousarray(np.tile(b34, (128, 1)))

    in_maps = []
    for c in range(N_CORES):
        lc = lpos[c * NLOC:(c + 1) * NLOC]
        degp = np.ones(W * 128, np.float32)
        degp[lc] = deg[c * NLOC:(c + 1) * NLOC]
        xp = np.empty((NLOC, x.shape[1]), np.float32)
        xp[lc] = x[c * NLOC:(c + 1) * NLOC]
        in_maps.append({
            "xT": np.ascontiguousarray(xp.T).astype(BF16),
            "idxs": idx_tiles[c],
            "S": np.ascontiguousarray(S_arr[c]),
            "degp": np.ascontiguousarray(degp.reshape(W, 128).T),
            "W1": W1.astype(BF16), "W2": W2.astype(BF16),
            "W34": W34.astype(BF16),
            "b1b": b1b, "b2b": b2b, "b34b": b34b,
            "id64": id64, "id128": id128,
        })

    _last["nc"] = nc
    _last["in_maps"] = in_maps
    res = run_bass_kernel_spmd(nc, in_maps, core_ids=list(range(N_CORES)))
    out = np.empty((N, 64), np.float32)
    for c in range(N_CORES):
        lc = lpos[c * NLOC:(c + 1) * NLOC]
        out[c * NLOC:(c + 1) * NLOC] = res.results[c]["out34"][lc]
    return out[:, :32].copy(), out[:, 32:].copy()


def kernel(**inputs):
    x = np.asarray(inputs["x"])
    ei = np.asarray(inputs["edge_index"])
    return _run(inputs, x.shape[0], ei.shape[1])


# revision 26
# speedup vs baseline: 1.1971x; 1.0008x over previous
"""3-layer GCN encoder (GCNConv x4, layers 3+4 fused) on 8 Trainium2 NeuronCores.

Strategy (graph/data parallel, per the edge-cut sharding hint):
  - Nodes partitioned contiguously across 8 cores (NLOC = N/8), degree-balanced
    window assignment (128 dst nodes per window) within each core.
  - Per layer: local transform T = h @ W on PE (bf16), rows scaled by dinv[src],
    written as a strided DRAM table (256B row stride; fp8 payload for layers
    1-2, bf16 for the final fused layer), AllGather -> full table per core.
  - Aggregation: per window group, dma_gather fetches each in-edge's source row
    (64B fp8 / 128B bf16 payload per descriptor -- raw-constructed gather
    instructions allow payload < 256B while the row stride stays 256B).
    Segment-sum runs on PE: per 128-slot tile, one matmul with a
    host-precomputed one-hot selection matrix S (fp8, SBUF-resident, shared by
    all three layers; zero columns at pad slots). Self-loop contributions come
    from the local stage tile via an identity matmul that also initializes the
    PSUM accumulator.
  - Epilogue per window: DVE (psum * dinv[dst] + bias), ReLU+bf16 cast, PE
    transpose into the next layer's feature-major input; final layer DMAs
    [nw, 64] = [mu | logstd] f32 rows to DRAM.
  - Gather indices are int16: rows >= 32768 are fetched by a second gather with
    the table base offset by 32768 rows (edges pre-split into lo/hi groups).

Self-contained: only needs numpy/ml_dtypes/concourse (container-installed).
"""

import os
import sys

if "/opt/trn_rl_repo" not in sys.path:
    sys.path.insert(0, "/opt/trn_rl_repo")

import numpy as np
import ml_dtypes

import concourse.bass as bass
import concourse.bacc as bacc
import concourse.mybir as mybir
import concourse.tile as tile
import concourse.ap_utils as ap_utils
from concourse.bass_utils import run_bass_kernel_spmd

BF16 = ml_dtypes.bfloat16
FP8 = ml_dtypes.float8_e4m3fn
F32 = mybir.dt.float32
BF = mybir.dt.bfloat16
E4 = mybir.dt.float8e4
I16 = mybir.dt.int16

N_CORES = 8
CHUNK = 32768  # int16 index range per gather call

_cache = {}
_last = {}


def last_run(trace=False, **kw):
    """Re-run the last compiled kernel/in_maps (optionally with NTFF tracing)."""
    if "nc" not in _last:
        return None
    return run_bass_kernel_spmd(_last["nc"], _last["in_maps"],
                                core_ids=list(range(N_CORES)), trace=trace, **kw)


def _balance(deg, N, NLOC, W):
    """Deal degree-sorted nodes round-robin into windows, per core."""
    lpos = np.empty(N, np.int64)
    caps = np.full(W, 128, np.int64)
    caps[W - 1] = NLOC - 128 * (W - 1)
    for c in range(N_CORES):
        dl = deg[c * NLOC:(c + 1) * NLOC]
        order_ = np.argsort(-dl, kind="stable")
        fill = np.zeros(W, np.int64)
        wi = 0
        pos = np.empty(NLOC, np.int64)
        for i in range(NLOC):
            while fill[wi % W] >= caps[wi % W]:
                wi += 1
            ww = wi % W
            pos[order_[i]] = ww * 128 + fill[ww]
            fill[ww] += 1
            wi += 1
        lpos[c * NLOC:(c + 1) * NLOC] = pos
    return lpos


def _prep(x, ei, N, NLOC, W, GRP):
    """Host-side edge bookkeeping: slot layout, gather indices, one-hot S."""
    src = ei[0].astype(np.int64)
    dst = ei[1].astype(np.int64)
    E = src.shape[0]
    deg = np.bincount(dst, minlength=N).astype(np.float32) + 1.0  # + self loop

    lpos = _balance(deg, N, NLOC, W)
    grow = (np.arange(N, dtype=np.int64) // NLOC) * NLOC + lpos

    srow = grow[src]
    c = dst // NLOC
    dloc = lpos[dst]
    w = dloc >> 7
    p = dloc & 127
    hi = (srow >= CHUNK).astype(np.int64)
    rel = (srow - hi * CHUNK).astype(np.int16)

    key = (c * W + w) * 2 + hi
    order = np.argsort(key, kind="stable")
    cnt = np.bincount(key, minlength=N_CORES * W * 2).reshape(N_CORES, W, 2)
    nt = (cnt.max(axis=0) + 127) // 128  # [W, 2] tiles per (window, chunk)

    # group/tile layout: per group of GRP windows, [all lo tiles | all hi tiles]
    tile_base = np.zeros((W, 2), np.int64)
    groups = []
    tb = 0
    for g0 in range(0, W, GRP):
        gn = min(GRP, W - g0)
        lo_base = tb
        for w_ in range(g0, g0 + gn):
            tile_base[w_, 0] = tb
            tb += nt[w_, 0]
        hi_base = tb
        for w_ in range(g0, g0 + gn):
            tile_base[w_, 1] = tb
            tb += nt[w_, 1]
        groups.append((g0, gn, int(lo_base), int(hi_base),
                       int(hi_base - lo_base), int(tb - hi_base)))
    T_tot = int(tb)
    P_tot = 128 * T_tot

    gstart = np.concatenate([[0], np.cumsum(cnt.reshape(-1))])
    pos_in_grp = np.arange(E) - gstart[key[order]]
    slot = tile_base[w[order], hi[order]] * 128 + pos_in_grp

    idx_arr = np.zeros((N_CORES, P_tot), np.int16)
    idx_arr[c[order], slot] = rel[order]
    S_arr = np.zeros((N_CORES, 128, P_tot), np.uint8)
    one = np.float32(1.0).astype(FP8).view(np.uint8)
    S_arr[c[order], slot % 128, (slot // 128) * 128 + p[order]] = one

    idx_tiles = []
    for cc in range(N_CORES):
        idx16 = np.ascontiguousarray(idx_arr[cc].reshape(P_tot // 16, 16).T)
        idx_tiles.append(np.ascontiguousarray(np.tile(idx16, (8, 1))))

    meta = {
        "N": N, "NLOC": NLOC, "W": W, "GRP": GRP,
        "nt": nt.tolist(), "tile_base": tile_base.tolist(),
        "groups": groups, "T_tot": T_tot, "P_tot": P_tot,
    }
    return meta, lpos, deg, idx_tiles, S_arr.view(FP8)


def _gather_raw(gp, out_ap, in_ap, idxs_ap, num_idxs, elem_size, elem_step):
    """dma_gather without the elem_size_bytes % 256 == 0 restriction.

    in_ap: DRAM [rows (stride elem_step elems), elem_size elems]
    out_ap: SBUF [128, num_idxs//128, elem_size]; idxs_ap: [128, num_idxs//16].
    """
    assert idxs_ap.dtype == mybir.dt.int16
    assert in_ap.dtype == out_ap.dtype
    assert in_ap.ap[0][0] == elem_step
    assert in_ap.ap[-1][1] == elem_size
    assert num_idxs % 128 == 0
    assert out_ap.ap[0][1] * out_ap.ap[1][1] == num_idxs
    assert ap_utils.ap_is_contiguous(out_ap.ap[1:])
    assert ap_utils.ap_is_contiguous(idxs_ap.ap[1:])
    stride_bytes = elem_step * mybir.dt.size(in_ap.dtype)
    stride_bytes_256 = stride_bytes // 256
    assert stride_bytes_256 * 256 == stride_bytes and 0 < stride_bytes_256 < 256
    return gp.add_instruction(
        mybir.InstDMAGatherAnt(
            name=gp.bass.get_next_instruction_name(),
            ins=[
                *gp.lower_ap_dma(in_ap, for_custom_bir_dma=True),
                gp.lower_ap(idxs_ap),
                gp.lower_val_access(gp.to_reg(num_idxs)),
            ],
            outs=[gp.lower_ap(out_ap)],
            transpose=False,
            num_idxs=num_idxs,
            elem_size=elem_size,
            stride_bytes_256=stride_bytes_256,
            gen_mode=0,
            single_packet=False,
            queue_num=0,
            sbuf_tokens_per_rank=0,
            sbuf_free_dim_per_rank=0,
            sbuf_free_dim_pad_per_rank=0,
            sbuf_byte_offset=0,
        )
    )


def _build(meta, solo):
    """Build the SPMD Bass program (solo=True: 1 core, no collectives)."""
    N, NLOC, W = meta["N"], meta["NLOC"], meta["W"]
    nt = meta["nt"]
    tile_base = meta["tile_base"]
    groups = meta["groups"]
    P_tot = meta["P_tot"]
    NTG_MAX = max(g[4] + g[5] for g in groups)
    AG = mybir.AluOpType
    RG = [list(range(N_CORES))]
    AF = mybir.ActivationFunctionType

    nc = bacc.Bacc("TRN2", target_bir_lowering=False, debug=False,
                   num_devices=1 if solo else N_CORES)

    xT_d = nc.dram_tensor("xT", (128, NLOC), BF, kind="ExternalInput")
    idxs_d = nc.dram_tensor("idxs", (128, P_tot // 16), I16,
                            kind="ExternalInput")
    S_d = nc.dram_tensor("S", (128, P_tot), E4, kind="ExternalInput")
    degp_d = nc.dram_tensor("degp", (128, W), F32, kind="ExternalInput")
    W1_d = nc.dram_tensor("W1", (128, 64), BF, kind="ExternalInput")
    W2_d = nc.dram_tensor("W2", (64, 64), BF, kind="ExternalInput")
    W34_d = nc.dram_tensor("W34", (64, 64), BF, kind="ExternalInput")
    b1_d = nc.dram_tensor("b1b", (128, 64), F32, kind="ExternalInput")
    b2_d = nc.dram_tensor("b2b", (128, 64), F32, kind="ExternalInput")
    b34_d = nc.dram_tensor("b34b", (128, 64), F32, kind="ExternalInput")
    id64_d = nc.dram_tensor("id64", (64, 64), BF, kind="ExternalInput")
    id128_d = nc.dram_tensor("id128", (128, 128), BF, kind="ExternalInput")
    out_d = nc.dram_tensor("out34", (NLOC, 64), F32, kind="ExternalOutput")

    ldt = [E4, E4, BF]                 # per-layer message dtype
    lcols = [256, 256, 128]            # table cols (256B row stride)
    tabl = [nc.dram_tensor(f"tabl{l}", (NLOC, lcols[l]), ldt[l],
                           kind="Internal") for l in range(3)]
    tabf = [nc.dram_tensor(f"tabf{l}", (N, lcols[l]), ldt[l], kind="Internal",
                           addr_space="Shared") for l in range(3)]

    with tile.TileContext(nc) as tc:
        with (
            tc.tile_pool(name="const", bufs=1) as const,
            tc.tile_pool(name="stg", bufs=2) as stg,
            tc.tile_pool(name="hTp", bufs=1) as hTp,
            tc.tile_pool(name="work", bufs=int(os.environ.get("K_WK", "8"))) as work,
            tc.tile_pool(name="gp", bufs=2) as gp,
            tc.tile_pool(name="psR", bufs=2, space="PSUM") as psR,
            tc.tile_pool(name="psA", bufs=int(os.environ.get("K_PSA", "4")),
                         space="PSUM") as psA,
        ):
            def cload(dram, shape, dt, tag):
                t = const.tile(shape, dt, tag=tag)
                nc.sync.dma_start(t[:], dram[:])
                return t

            W1t = cload(W1_d, [128, 64], BF, "W1t")
            W2t = cload(W2_d, [64, 64], BF, "W2t")
            W34t = cload(W34_d, [64, 64], BF, "W34t")
            b1t = cload(b1_d, [128, 64], F32, "b1t")
            b2t = cload(b2_d, [128, 64], F32, "b2t")
            b34t = cload(b34_d, [128, 64], F32, "b34t")
            id128 = cload(id128_d, [128, 128], BF, "id128")

            degp = cload(degp_d, [128, W], F32, "degp")
            sqp = const.tile([128, W], F32, tag="sqp")
            nc.scalar.activation(sqp[:], degp[:], AF.Sqrt)
            dinvp = const.tile([128, W], F32, tag="dinvp")
            nc.vector.reciprocal(dinvp[:], sqp[:])

            # big constant loads from the (early-idle) ACT queue; S in pieces
            # so small transform DMAs can interleave on the DMA engines
            idxs = const.tile([128, P_tot // 16], I16, tag="idxs")
            nc.scalar.dma_start(idxs[:], idxs_d[:])
            S = const.tile([128, P_tot], E4, tag="S")
            SCH = 1680
            for c0_ in range(0, P_tot, SCH):
                cn_ = min(SCH, P_tot - c0_)
                nc.scalar.dma_start(S[:, c0_:c0_ + cn_], S_d[:, c0_:c0_ + cn_])

            NCH = (NLOC + 511) // 512

            def make_transform(l, hT, K, Wt):
                """Returns (stage, emit_chunk, finish). emit_chunk(j): compute
                T = h @ W for nodes [512j, 512j+cn), dinv[src]-scaled and
                transposed into stage, table rows written strided to DRAM."""
                stage = stg.tile([128, W * 64], ldt[l], tag="stage")

                def emit(j):
                    c0 = 512 * j
                    cn = min(512, NLOC - c0)
                    if hT is None:
                        xc = work.tile([128, 512], BF, tag="xc")
                        nc.sync.dma_start(xc[:, :cn], xT_d[:, c0:c0 + cn])
                        hsrc = xc
                    else:
                        hsrc = hT[j]
                    w0, w1 = c0 // 128, min((c0 + cn + 127) // 128, W)
                    for w in range(w0, w1):
                        off = w * 128 - c0
                        nw = min(128, NLOC - w * 128)
                        # node-major rows directly: out[node, f] via
                        # lhsT = feature-major h slice, rhs = W
                        psW = psR.tile([128, 64], F32, tag="psW")
                        nc.tensor.matmul(psW[:nw, :], hsrc[:K, off:off + nw],
                                         Wt[:K, :], start=True, stop=True)
                        if w % 2 == 0:
                            nc.scalar.activation(
                                stage[:nw, 64 * w:64 * w + 64], psW[:nw, :],
                                AF.Copy, scale=dinvp[:nw, w:w + 1])
                        else:
                            nc.vector.tensor_scalar(
                                out=stage[:nw, 64 * w:64 * w + 64],
                                in0=psW[:nw, :], scalar1=dinvp[:nw, w:w + 1],
                                scalar2=None, op0=AG.mult)
                        if nw < 128:
                            nc.vector.memset(stage[nw:, 64 * w:64 * w + 64], 0.0)
                    if cn == 512:
                        nc.sync.dma_start(
                            tabl[l][c0:c0 + cn, 0:64].rearrange(
                                "(w p) e -> p w e", p=128),
                            stage[:, 64 * w0:64 * w1])
                    else:
                        for w in range(w0, w1):
                            nw = min(128, NLOC - w * 128)
                            nc.sync.dma_start(
                                tabl[l][w * 128:w * 128 + nw, 0:64],
                                stage[:nw, 64 * w:64 * w + 64])

                def finish():
                    if not solo:
                        nc.gpsimd.collective_compute(
                            "AllGather", AG.bypass, replica_groups=RG,
                            ins=[tabl[l][:].opt()], outs=[tabf[l][:].opt()])

                return stage, emit, finish

            def aggregate(l, stage, bias_t, relu, hT_next, nxt=None):
                """nxt = (emit, finish) of the next layer's transform; chunks
                are emitted as soon as their source windows are done."""
                in_lo = tabf[l][0:CHUNK, 0:64]
                in_hi = tabf[l][CHUNK:N, 0:64]
                emitted = 0
                ost = [None]
                for (g0, gn, lo_base, hi_base, ntg_lo, ntg_hi) in groups:
                    g = gp.tile([128, NTG_MAX, 64], ldt[l], tag="g")
                    if ntg_lo:
                        _gather_raw(
                            nc.gpsimd, g[:, :ntg_lo, :], in_lo,
                            idxs[:, lo_base * 8:(lo_base + ntg_lo) * 8],
                            ntg_lo * 128, 64, lcols[l])
                    if ntg_hi:
                        _gather_raw(
                            nc.gpsimd, g[:, ntg_lo:ntg_lo + ntg_hi, :], in_hi,
                            idxs[:, hi_base * 8:(hi_base + ntg_hi) * 8],
                            ntg_hi * 128, 64, lcols[l])
                    for w in range(g0, g0 + gn):
                        nw = min(128, NLOC - w * 128)
                        n_lo, n_hi = nt[w][0], nt[w][1]
                        ps = psA.tile([128, 64], F32, tag="psA")
                        nc.tensor.matmul(ps[:], id128[:, :],
                                         stage[:, 64 * w:64 * w + 64],
                                         start=True, stop=(n_lo + n_hi == 0))
                        for t in range(n_lo):
                            tb_ = tile_base[w][0] + t
                            nc.tensor.matmul(
                                ps[:], S[:, tb_ * 128:tb_ * 128 + 128],
                                g[:, tb_ - lo_base, :], start=False,
                                stop=(n_hi == 0 and t == n_lo - 1))
                        for t in range(n_hi):
                            tb_ = tile_base[w][1] + t
                            nc.tensor.matmul(
                                ps[:], S[:, tb_ * 128:tb_ * 128 + 128],
                                g[:, ntg_lo + tb_ - hi_base, :], start=False,
                                stop=(t == n_hi - 1))
                        if relu:
                            hw_ = work.tile([128, 64], F32, tag="hw")
                            nc.vector.scalar_tensor_tensor(
                                out=hw_[:], in0=ps[:], scalar=dinvp[:, w:w + 1],
                                in1=bias_t[:], op0=AG.mult, op1=AG.add)
                            hwb = work.tile([128, 64], BF, tag="hwb")
                            nc.vector.tensor_scalar(
                                out=hwb[:], in0=hw_[:], scalar1=0.0,
                                scalar2=None, op0=AG.max)
                            pt2 = psR.tile([128, 128], BF, tag="ptr")
                            nc.tensor.transpose(pt2[:64, :nw], hwb[:nw, :],
                                                id128[:nw, :nw])
                            j, r0 = (w * 128) // 512, (w * 128) % 512
                            nc.scalar.copy(hT_next[j][:, r0:r0 + nw],
                                           pt2[:64, :nw])
                        else:
                            # stage 4 windows into a chunk tile, flush as one
                            # strided DMA (fewer HWDGE ops)
                            j = (w * 128) // 512
                            if w % 4 == 0:
                                ost[0] = work.tile([128, 256], F32,
                                                   name="ost", tag="ost")
                            nc.vector.scalar_tensor_tensor(
                                out=ost[0][:, (w % 4) * 64:(w % 4) * 64 + 64],
                                in0=ps[:], scalar=dinvp[:, w:w + 1],
                                in1=bias_t[:], op0=AG.mult, op1=AG.add)
                            if w == W - 1 or w % 4 == 3:
                                c0 = 512 * j
                                cn = min(512, NLOC - c0)
                                if cn == 512:
                                    nc.sync.dma_start(
                                        out_d[c0:c0 + cn, :].rearrange(
                                            "(w p) e -> p w e", p=128),
                                        ost[0][:, :256])
                                else:
                                    for w_ in range(4 * j, W):
                                        nw_ = min(128, NLOC - w_ * 128)
                                        nc.sync.dma_start(
                                            out_d[w_ * 128:w_ * 128 + nw_, :],
                                            ost[0][:nw_, (w_ % 4) * 64:
                                                   (w_ % 4) * 64 + 64])
                    if nxt is not None:
                        done = g0 + gn  # windows completed
                        while (emitted < NCH
                               and (emitted + 1) * 512 <= done * 128):
                            nxt[0](emitted)
                            emitted += 1
                if nxt is not None:
                    while emitted < NCH:
                        nxt[0](emitted)
                        emitted += 1
                    nxt[1]()

            # hT3 aliases hT2: transform-2 chunk j (the only hT2[j] reader)
            # strictly precedes any L2-agg epilogue write (L2 agg waits on the
            # complete layer-2 table), so reuse is hazard-free.
            hT2 = [hTp.tile([64, 512], BF, name=f"hT2_{j}", tag=f"hT2_{j}")
                   for j in range(NCH)]
            hT3 = hT2
            PH = int(os.environ.get("K_PHASES", "9"))
            st0, em0, fin0 = make_transform(0, None, 128, W1t)
            st1, em1, fin1 = make_transform(1, hT2, 64, W2t)
            st2, em2, fin2 = make_transform(2, hT3, 64, W34t)
            if PH >= 1:
                for j in range(NCH):
                    em0(j)
                fin0()
            if PH >= 3:
                aggregate(0, st0, b1t, True, hT2,
                          nxt=(em1, fin1) if PH >= 4 else None)
            if PH >= 6:
                aggregate(1, st1, b2t, True, hT3,
                          nxt=(em2, fin2) if PH >= 7 else None)
            if PH >= 9:
                aggregate(2, st2, b34t, False, None)

    nc.compile()
    return nc


def _run(inputs, N, E):
    NLOC = N // N_CORES
    W = (NLOC + 127) // 128
    GRP = int(os.environ.get("K_GRP", "7"))

    x = np.asarray(inputs["x"], np.float32)
    ei = np.asarray(inputs["edge_index"], np.int64)
    W1 = np.asarray(inputs["W1"], np.float32)
    b1 = np.asarray(inputs["b1"], np.float32)
    W2 = np.asarray(inputs["W2"], np.float32)
    b2 = np.asarray(inputs["b2"], np.float32)
    Wmu = np.asarray(inputs["Wmu"], np.float32)
    bmu = np.asarray(inputs["bmu"], np.float32)
    Wls = np.asarray(inputs["Wls"], np.float32)
    bls = np.asarray(inputs["bls"], np.float32)

    meta, lpos, deg, idx_tiles, S_arr = _prep(x, ei, N, NLOC, W, GRP)

    key = (N, NLOC, W, meta["T_tot"], os.environ.get("K_PHASES", "9"))
    if key not in _cache:
        _cache[key] = _build(meta, solo=False)
    nc = _cache[key]

    W34 = np.concatenate([Wmu, Wls], axis=1)
    b34 = np.concatenate([bmu, bls])
    id64 = np.eye(64, dtype=np.float32).astype(BF16)
    id128 = np.eye(128, dtype=np.float32).astype(BF16)
    b1b = np.ascontiguousarray(np.tile(b1, (128, 1)))
    b2b = np.ascontiguousarray(np.tile(b2, (128, 1)))
    b34b = np.ascontiguousarray(np.tile(b34, (128, 1)))

    in_maps = []
    for c in range(N_CORES):
        lc = lpos[c * NLOC:(c + 1) * NLOC]
        degp = np.ones(W * 128, np.float32)
        degp[lc] = deg[c * NLOC:(c + 1) * NLOC]
        xp = np.empty((NLOC, x.shape[1]), np.float32)
        xp[lc] = x[c * NLOC:(c + 1) * NLOC]
        in_maps.append({
            "xT": np.ascontiguousarray(xp.T).astype(BF16),
            "idxs": idx_tiles[c],
            "S": np.ascontiguousarray(S_arr[c]),
            "degp": np.ascontiguousarray(degp.reshape(W, 128).T),
            "W1": W1.astype(BF16), "W2": W2.astype(BF16),
            "W34": W34.astype(BF16),
            "b1b": b1b, "b2b": b2b, "b34b": b34b,
            "id64": id64, "id128": id128,
        })

    _last["nc"] = nc
    _last["in_maps"] = in_maps
    res = run_bass_kernel_spmd(nc, in_maps, core_ids=list(range(N_CORES)))
    out = np.empty((N, 64), np.float32)
    for c in range(N_CORES):
        lc = lpos[c * NLOC:(c + 1) * NLOC]
        out[c * NLOC:(c + 1) * NLOC] = res.results[c]["out34"][lc]
    return out[:, :32].copy(), out[:, 32:].copy()


def kernel(**inputs):
    x = np.asarray(inputs["x"])
    ei = np.asarray(inputs["edge_index"])
    return _run(inputs, x.shape[0], ei.shape[1])
